# revision 3
# baseline (speedup 1.0000x reference)
"""Single-head causal attention forward on 8 TRN2 NeuronCores.

Problem: x [8, 2048, 1024] f32, Wq/Wk/Wv [128, 1024] f32.
  q/k/v = x @ W.T ; S = q k^T / sqrt(128) causal ; out = softmax(S) v.

Sharding: data-parallel, one batch element per core (8 cores).
Inside each core a flash-style blocked attention over 512-token chunks:
  - host pre-transposes x[b] into a chunk-major layout [chunk, p, cc, t] so
    the contraction dim (c) lands on SBUF partitions and every DMA piece is
    one contiguous run per partition.
  - qT/kT/vT [h=128, t] via W-stationary matmuls (N=512, weight loads
    hidden); V natural [t, h] via PE transposes of vT into a slotted PSUM
    bank.
  - S^T[j, q] tiles computed full-width (512) in PAIRS into [128,1024] PSUM
    tiles (2 banks); ONE exp ACTIVATE per pair halves ScalarE call overhead.
    A ones-column appended to V makes the PV matmul also produce the softmax
    denominators, so no partition-direction reduction is ever needed.
  - causal masking of the 16 diagonal 128x128 blocks is done by ACCUMULATING
    -1024 into the strictly-masked entries via one extra matmul
    (stationary=eye, rhs=-1024*strict_lower) before the exp: exp maps them
    to ~0, so no DVE mask-multiply and no separate masked tile.
  - PV accumulators live in 3 column-slots of a single PSUM bank (chains
    are strictly sequential on the PE so slot sharing is safe).
  - chunk qc's PV chains are emitted inside chunk qc+1 so PE never waits
    on exp latency except at the tail.
"""

import os
import sys

for _p in ("/opt/trn_rl_repo",):
    if _p not in sys.path and os.path.isdir(_p):
        sys.path.append(_p)

import numpy as np

B, T, D, H = 8, 2048, 1024, 128
CH = 512          # token chunk (free dim of S^T tiles)
NCH = T // CH     # 4 chunks
CC = D // 128     # 8 contraction sub-tiles
NT = T // 128     # 16 token tiles
SCALE = 1.0 / np.sqrt(np.float32(H))
MASKV = -1024.0   # additive pre-scale mask; SCALE*1024 ~ 90 -> exp ~ 1e-39

PROJ_DT = os.environ.get("KERNEL_PROJ_DT", "bfloat16")
ATT_DT = os.environ.get("KERNEL_ATT_DT", "bfloat16")
NWU = int(os.environ.get("KERNEL_NWU", "4"))

_CACHE = {}


def _build():
    import concourse.bacc as bacc
    import concourse.mybir as mybir
    import concourse.tile as tile

    dt = mybir.dt
    p_dt = getattr(dt, PROJ_DT)
    a_dt = getattr(dt, ATT_DT)

    nc = bacc.Bacc(None)
    xh = nc.declare_dram_parameter("xh", [NCH, 128, CC, CH], p_dt, isOutput=False)
    wqT = nc.declare_dram_parameter("wqT", [128, CC, H], p_dt, isOutput=False)
    wkT = nc.declare_dram_parameter("wkT", [128, CC, H], p_dt, isOutput=False)
    wvT = nc.declare_dram_parameter("wvT", [128, CC, H], p_dt, isOutput=False)
    msk = nc.declare_dram_parameter("msk", [128, 128], a_dt, isOutput=False)
    eye = nc.declare_dram_parameter("eye", [128, 128], a_dt, isOutput=False)
    out = nc.declare_dram_parameter("out", [T, H], dt.float32, isOutput=True)

    with tile.TileContext(nc) as tc:
        with (
            tc.tile_pool(name="singles", bufs=1) as singles,
            tc.tile_pool(name="xp", bufs=2) as xp,
            tc.tile_pool(name="qtp", bufs=2) as qtp,
            tc.tile_pool(name="ktp", bufs=4) as ktp,
            tc.tile_pool(name="vtp", bufs=2) as vtp,
            tc.tile_pool(name="ptp", bufs=15) as ptp,
            tc.tile_pool(name="outp", bufs=4) as outp,
            tc.tile_pool(name="recp", bufs=4) as recp,
            tc.tile_pool(name="psq", bufs=2, space="PSUM") as psq,
            tc.tile_pool(name="pss", bufs=2, space="PSUM") as pss,
            tc.tile_pool(name="psv", bufs=1, space="PSUM") as psv,
            tc.tile_pool(name="pst", bufs=1, space="PSUM") as pst,
        ):
            # PE warmup on zeroed tiles (HAM clock release). memsets on
            # GpSimd so the chain starts right after the framework barrier,
            # while the first x/weight DMAs are still in flight.
            wu_a = singles.tile([128, 128], a_dt)
            wu_b = singles.tile([128, CH], a_dt)
            wu_a_ap, wu_b_ap = wu_a[:], wu_b[:]
            if ATT_DT == "float32r":
                wu_a_ap = wu_a_ap.bitcast(dt.float32)
                wu_b_ap = wu_b_ap.bitcast(dt.float32)
            nc.gpsimd.memset(wu_a_ap, 0.0)
            nc.gpsimd.memset(wu_b_ap, 0.0)
            wu_ps = psq.tile([128, CH], dt.float32, tag="pq")
            for i in range(NWU):
                nc.tensor.matmul(
                    wu_ps[:], wu_a[:], wu_b[:],
                    start=(i == 0), stop=(i == NWU - 1),
                )

            # --- weights / constants: priority order on the gpsimd queue ---
            wq_sb = singles.tile([128, CC, H], p_dt)
            wk_sb = singles.tile([128, CC, H], p_dt)
            wv_sb = singles.tile([128, CC, H], p_dt)
            eye_sb = singles.tile([128, 128], a_dt)
            msk_sb = singles.tile([128, 128], a_dt)
            nc.gpsimd.dma_start(out=wq_sb[:, 0:4, :], in_=wqT[:, 0:4, :])
            nc.gpsimd.dma_start(out=wq_sb[:, 4:CC, :], in_=wqT[:, 4:CC, :])
            nc.gpsimd.dma_start(out=eye_sb[:], in_=eye[:])
            nc.gpsimd.dma_start(out=msk_sb[:], in_=msk[:])
            nc.gpsimd.dma_start(out=wk_sb[:], in_=wkT[:])
            nc.gpsimd.dma_start(out=wv_sb[:], in_=wvT[:])

            # V' = [V | 1]; ones columns written once
            v_sb = singles.tile([128, NT, H + 4], a_dt)
            ones_ap = v_sb[:, :, H : H + 2]
            if ATT_DT == "float32r":
                ones_ap = ones_ap.bitcast(dt.float32)
            nc.vector.memset(ones_ap, 1.0)

            # persistent slotted PSUM banks
            pv_acc = psv.tile([128, 512], dt.float32)   # 3 slots x 130
            tr_acc = pst.tile([128, 512], a_dt)         # 4 slots x 128

            kt_tiles = []
            pts_all = []   # per chunk: list of pt pair tiles [128, 2*CH]

            def emit_chains(qc):
                """PV chains + normalize + store for q-chunk qc."""
                pts_c = pts_all[qc]
                for ti in range(4):
                    qi = qc * 4 + ti
                    slot = qi % 3
                    ops = pv_acc[:, slot * 130 : slot * 130 + 130]
                    for j2 in range(qi + 1):
                        pt_pair = pts_c[j2 // 2]
                        c0 = (j2 % 2) * CH + ti * 128
                        nc.tensor.matmul(
                            ops[:, 0 : H + 2],
                            pt_pair[:, c0 : c0 + 128],
                            v_sb[:, j2, 0 : H + 2],
                            start=(j2 == 0), stop=(j2 == qi),
                        )
                    rec = recp.tile([128, 1], dt.float32)
                    nc.vector.reciprocal(rec[:], ops[:, H : H + 1])
                    ob = outp.tile([128, H], dt.float32)
                    nc.vector.tensor_scalar_mul(ob[:], ops[:, 0:H], rec[:])
                    eng = nc.sync if (qi % 2 == 0) else nc.gpsimd
                    eng.dma_start(
                        out=out[qi * 128 : (qi + 1) * 128, :], in_=ob[:]
                    )

            for qc in range(NCH):
                # x chunk [128, CC, CH]; early chunks in quarters so the
                # first projection matmuls start as soon as possible.
                xt = xp.tile([128, CC, CH], p_dt)
                step = 2 if qc == 0 else 4
                for g0 in range(0, CC, step):
                    nc.sync.dma_start(
                        out=xt[:, g0 : g0 + step, :],
                        in_=xh[qc, :, g0 : g0 + step, :],
                    )

                # --- qT, kT: [h=128, CH] ---
                qps = psq.tile([128, CH], dt.float32, tag="pq")
                for cc in range(CC):
                    nc.tensor.matmul(
                        qps[:], wq_sb[:, cc, :], xt[:, cc, :],
                        start=(cc == 0), stop=(cc == CC - 1),
                    )
                qt = qtp.tile([128, CH], a_dt)
                nc.vector.tensor_copy(qt[:], qps[:])

                kps = psq.tile([128, CH], dt.float32, tag="pq")
                for cc in range(CC):
                    nc.tensor.matmul(
                        kps[:], wk_sb[:, cc, :], xt[:, cc, :],
                        start=(cc == 0), stop=(cc == CC - 1),
                    )
                kt = ktp.tile([128, CH], a_dt)
                nc.vector.tensor_copy(kt[:], kps[:])
                kt_tiles.append(kt)

                # --- S^T pairs: two full-width j-tiles per [128,1024] PSUM
                # tile, one exp per pair. Diagonal blocks get -1024 added to
                # their strictly-masked entries via an extra matmul. ---
                pts_c = []
                for p in range(2 * qc + 2):
                    sp = pss.tile([128, 2 * CH], dt.float32)
                    pt = ptp.tile([128, 2 * CH], a_dt)
                    for hh in range(2):
                        jt = 2 * p + hh
                        kt_src = kt_tiles[jt // 4]
                        diag = jt >= qc * 4
                        nc.tensor.matmul(
                            sp[:, hh * CH : (hh + 1) * CH],
                            kt_src[:, (jt % 4) * 128 : (jt % 4 + 1) * 128],
                            qt[:],
                            start=True, stop=not diag,
                        )
                        if diag:
                            ti = jt - qc * 4
                            b0 = hh * CH + ti * 128
                            nc.tensor.matmul(
                                sp[:, b0 : b0 + 128],
                                eye_sb[:], msk_sb[:],
                                start=False, stop=True,
                            )
                    nc.scalar.activation(
                        pt[:], sp[:], mybir.ActivationFunctionType.Exp,
                        scale=float(SCALE),
                    )
                    pts_c.append(pt)
                pts_all.append(pts_c)

                # previous chunk's PV chains: their exps are long done, so
                # the PE streams them while this chunk's exps run.
                if qc > 0:
                    emit_chains(qc - 1)

                # --- vT + V natural via PE transposes into slotted bank ---
                vps = psq.tile([128, CH], dt.float32, tag="pq")
                for cc in range(CC):
                    nc.tensor.matmul(
                        vps[:], wv_sb[:, cc, :], xt[:, cc, :],
                        start=(cc == 0), stop=(cc == CC - 1),
                    )
                vt = vtp.tile([128, CH], a_dt)
                nc.vector.tensor_copy(vt[:], vps[:])

                for ti in range(4):
                    jt = qc * 4 + ti
                    dst = tr_acc[:, ti * 128 : (ti + 1) * 128]
                    nc.tensor.transpose(
                        dst, vt[:, ti * 128 : (ti + 1) * 128], eye_sb[:]
                    )
                    nc.vector.tensor_copy(v_sb[:, jt, 0:H], dst)

            emit_chains(NCH - 1)

    nc.compile()
    return nc


def _get_nc():
    if "nc" not in _CACHE:
        _CACHE["nc"] = _build()
    return _CACHE["nc"]


def _np_dt(name):
    if name == "bfloat16":
        import ml_dtypes

        return ml_dtypes.bfloat16
    return np.float32


def _in_maps(x, Wq, Wk, Wv):
    pdt = _np_dt(PROJ_DT)
    adt = _np_dt(ATT_DT)

    def _wprep(W):
        # W [H, D] -> [128p, CC, H] with per-partition-contiguous rows
        WT = np.asarray(W, dtype=np.float32).T.reshape(CC, 128, H)
        return np.ascontiguousarray(WT.transpose(1, 0, 2)).astype(pdt)

    wq, wk, wv = _wprep(Wq), _wprep(Wk), _wprep(Wv)
    # msk[j, q] = MASKV where q < j (strictly masked in the diagonal block)
    msk = (MASKV * np.tril(np.ones((128, 128), dtype=np.float32), -1)).astype(adt)
    eye = np.eye(128, dtype=np.float32).astype(adt)
    x = np.asarray(x, dtype=np.float32)
    maps = []
    for b in range(B):
        # [qc, p, cc, t]: per (qc, p) a contiguous CC*CH run
        xh = np.ascontiguousarray(
            x[b].T.reshape(CC, 128, NCH, CH).transpose(2, 1, 0, 3)
        ).astype(pdt)
        maps.append(
            {
                "xh": xh, "wqT": wq, "wkT": wk, "wvT": wv,
                "msk": msk, "eye": eye,
            }
        )
    return maps


def kernel(x, Wq, Wk, Wv):
    from concourse.bass_utils import run_bass_kernel_spmd

    nc = _get_nc()
    res = run_bass_kernel_spmd(nc, _in_maps(x, Wq, Wk, Wv), core_ids=list(range(B)))
    return np.stack([res.results[b]["out"] for b in range(B)]).astype(np.float32)


# revision 6
# speedup vs baseline: 1.1928x; 1.1928x over previous
"""Single-head causal attention forward on 8 TRN2 NeuronCores.

Problem: x [8, 2048, 1024] f32, Wq/Wk/Wv [128, 1024] f32.
  q/k/v = x @ W.T ; S = q k^T / sqrt(128) causal ; out = softmax(S) v.

Sharding: data-parallel, one batch element per core (8 cores).
Inside each core a flash-style blocked attention over 512-token chunks:
  - host pre-transposes x[b] into a chunk-major layout [chunk, p, cc, t] so
    the contraction dim (c) lands on SBUF partitions and every DMA piece is
    one contiguous run per partition.
  - qT/kT/vT [h=128, t] via W-stationary matmuls (N=512, weight loads
    hidden); V natural [t, h] via PE transposes of vT into a slotted PSUM
    bank.
  - S^T[j, q] tiles computed full-width (512) in PAIRS into [128,1024] PSUM
    tiles (2 banks); ONE exp ACTIVATE per pair halves ScalarE call overhead.
    A ones-column appended to V makes the PV matmul also produce the softmax
    denominators, so no partition-direction reduction is ever needed.
  - causal masking of the 16 diagonal 128x128 blocks is done by ACCUMULATING
    -1024 into the strictly-masked entries via one extra matmul
    (stationary=eye, rhs=-1024*strict_lower) before the exp: exp maps them
    to ~0, so no DVE mask-multiply and no separate masked tile.
  - PV accumulators live in 3 column-slots of a single PSUM bank (chains
    are strictly sequential on the PE so slot sharing is safe).
  - chunk qc's PV chains are emitted inside chunk qc+1 so PE never waits
    on exp latency except at the tail.
"""

import os
import sys

for _p in ("/opt/trn_rl_repo",):
    if _p not in sys.path and os.path.isdir(_p):
        sys.path.append(_p)

import numpy as np

B, T, D, H = 8, 2048, 1024, 128
CH = 512          # token chunk (free dim of S^T tiles)
NCH = T // CH     # 4 chunks
CC = D // 128     # 8 contraction sub-tiles
NT = T // 128     # 16 token tiles
SCALE = 1.0 / np.sqrt(np.float32(H))
MASKV = -1024.0   # additive pre-scale mask; SCALE*1024 ~ 90 -> exp ~ 1e-39

PROJ_DT = os.environ.get("KERNEL_PROJ_DT", "bfloat16")
ATT_DT = os.environ.get("KERNEL_ATT_DT", "bfloat16")
NWU = int(os.environ.get("KERNEL_NWU", "6"))

_CACHE = {}


def _build():
    import concourse.bacc as bacc
    import concourse.mybir as mybir
    import concourse.tile as tile

    dt = mybir.dt
    p_dt = getattr(dt, PROJ_DT)
    a_dt = getattr(dt, ATT_DT)

    nc = bacc.Bacc(None)
    xh = nc.declare_dram_parameter("xh", [NCH, 128, CC, CH], p_dt, isOutput=False)
    wqT = nc.declare_dram_parameter("wqT", [128, CC, H], p_dt, isOutput=False)
    wkT = nc.declare_dram_parameter("wkT", [128, CC, H], p_dt, isOutput=False)
    wvT = nc.declare_dram_parameter("wvT", [128, CC, H], p_dt, isOutput=False)
    msk = nc.declare_dram_parameter("msk", [128, 128], a_dt, isOutput=False)
    eye = nc.declare_dram_parameter("eye", [128, 128], a_dt, isOutput=False)
    out = nc.declare_dram_parameter("out", [T, H], dt.float32, isOutput=True)

    with tile.TileContext(nc) as tc:
        with (
            tc.tile_pool(name="singles", bufs=1) as singles,
            tc.tile_pool(name="xp", bufs=2) as xp,
            tc.tile_pool(name="qtp", bufs=2) as qtp,
            tc.tile_pool(name="ktp", bufs=4) as ktp,
            tc.tile_pool(name="vtp", bufs=2) as vtp,
            tc.tile_pool(name="ptp", bufs=15) as ptp,
            tc.tile_pool(name="outp", bufs=4) as outp,
            tc.tile_pool(name="recp", bufs=4) as recp,
            tc.tile_pool(name="psq", bufs=2, space="PSUM") as psq,
            tc.tile_pool(name="pss", bufs=2, space="PSUM") as pss,
            tc.tile_pool(name="psv", bufs=1, space="PSUM") as psv,
            tc.tile_pool(name="pst", bufs=1, space="PSUM") as pst,
        ):
            # PE warmup on zeroed tiles (HAM clock release). memsets on
            # GpSimd so the chain starts right after the framework barrier,
            # while the first x/weight DMAs are still in flight.
            wu_a = singles.tile([128, 128], a_dt)
            wu_b = singles.tile([128, CH], a_dt)
            wu_a_ap, wu_b_ap = wu_a[:], wu_b[:]
            if ATT_DT == "float32r":
                wu_a_ap = wu_a_ap.bitcast(dt.float32)
                wu_b_ap = wu_b_ap.bitcast(dt.float32)
            nc.gpsimd.memset(wu_a_ap, 0.0)
            nc.gpsimd.memset(wu_b_ap, 0.0)
            wu_ps = psq.tile([128, CH], dt.float32, tag="pq")
            for i in range(NWU):
                nc.tensor.matmul(
                    wu_ps[:], wu_a[:], wu_b[:],
                    start=(i == 0), stop=(i == NWU - 1),
                )

            # --- weights / constants: one queue per tensor so the triggers
            # don't serialize and wk/wv aren't stuck behind the x stream ---
            wq_sb = singles.tile([128, CC, H], p_dt)
            wk_sb = singles.tile([128, CC, H], p_dt)
            wv_sb = singles.tile([128, CC, H], p_dt)
            eye_sb = singles.tile([128, 128], a_dt)
            msk_sb = singles.tile([128, 128], a_dt)
            nc.gpsimd.dma_start(out=wq_sb[:], in_=wqT[:])
            nc.scalar.dma_start(out=wk_sb[:], in_=wkT[:])
            nc.scalar.dma_start(out=wv_sb[:], in_=wvT[:])
            nc.gpsimd.dma_start(out=eye_sb[:], in_=eye[:])
            nc.gpsimd.dma_start(out=msk_sb[:], in_=msk[:])

            # V' = [V | 1]; ones columns written once
            v_sb = singles.tile([128, NT, H + 4], a_dt)
            ones_ap = v_sb[:, :, H : H + 2]
            if ATT_DT == "float32r":
                ones_ap = ones_ap.bitcast(dt.float32)
            nc.vector.memset(ones_ap, 1.0)

            # persistent slotted PSUM banks
            pv_acc = psv.tile([128, 512], dt.float32)   # 3 slots x 130
            tr_acc = pst.tile([128, 512], a_dt)         # 4 slots x 128

            kt_tiles = []
            pts_all = []   # per chunk: list of pt pair tiles [128, 2*CH]

            def emit_chains(qc):
                """PV chains + normalize + store for q-chunk qc."""
                pts_c = pts_all[qc]
                for ti in range(4):
                    qi = qc * 4 + ti
                    slot = qi % 3
                    ops = pv_acc[:, slot * 130 : slot * 130 + 130]
                    for j2 in range(qi + 1):
                        pt_pair = pts_c[j2 // 2]
                        c0 = (j2 % 2) * CH + ti * 128
                        nc.tensor.matmul(
                            ops[:, 0 : H + 2],
                            pt_pair[:, c0 : c0 + 128],
                            v_sb[:, j2, 0 : H + 2],
                            start=(j2 == 0), stop=(j2 == qi),
                        )
                    rec = recp.tile([128, 1], dt.float32)
                    nc.vector.reciprocal(rec[:], ops[:, H : H + 1])
                    ob = outp.tile([128, H], dt.float32)
                    nc.vector.tensor_scalar_mul(ob[:], ops[:, 0:H], rec[:])
                    eng = nc.sync if (qi % 2 == 0) else nc.gpsimd
                    eng.dma_start(
                        out=out[qi * 128 : (qi + 1) * 128, :], in_=ob[:]
                    )

            for qc in range(NCH):
                # x chunk [128, CC, CH]; early chunks in quarters so the
                # first projection matmuls start as soon as possible.
                xt = xp.tile([128, CC, CH], p_dt)
                step = 2 if qc == 0 else 4
                for g0 in range(0, CC, step):
                    nc.sync.dma_start(
                        out=xt[:, g0 : g0 + step, :],
                        in_=xh[qc, :, g0 : g0 + step, :],
                    )

                # --- qT, kT: [h=128, CH] ---
                qps = psq.tile([128, CH], dt.float32, tag="pq")
                for cc in range(CC):
                    nc.tensor.matmul(
                        qps[:], wq_sb[:, cc, :], xt[:, cc, :],
                        start=(cc == 0), stop=(cc == CC - 1),
                    )
                qt = qtp.tile([128, CH], a_dt)
                nc.vector.tensor_copy(qt[:], qps[:])

                kps = psq.tile([128, CH], dt.float32, tag="pq")
                for cc in range(CC):
                    nc.tensor.matmul(
                        kps[:], wk_sb[:, cc, :], xt[:, cc, :],
                        start=(cc == 0), stop=(cc == CC - 1),
                    )
                kt = ktp.tile([128, CH], a_dt)
                nc.vector.tensor_copy(kt[:], kps[:])
                kt_tiles.append(kt)

                # --- S^T pairs: two full-width j-tiles per [128,1024] PSUM
                # tile, one exp per pair. Diagonal blocks get -1024 added to
                # their strictly-masked entries via an extra matmul. ---
                pts_c = []
                for p in range(2 * qc + 2):
                    sp = pss.tile([128, 2 * CH], dt.float32)
                    pt = ptp.tile([128, 2 * CH], a_dt)
                    for hh in range(2):
                        jt = 2 * p + hh
                        kt_src = kt_tiles[jt // 4]
                        diag = jt >= qc * 4
                        nc.tensor.matmul(
                            sp[:, hh * CH : (hh + 1) * CH],
                            kt_src[:, (jt % 4) * 128 : (jt % 4 + 1) * 128],
                            qt[:],
                            start=True, stop=not diag,
                        )
                        if diag:
                            ti = jt - qc * 4
                            b0 = hh * CH + ti * 128
                            nc.tensor.matmul(
                                sp[:, b0 : b0 + 128],
                                eye_sb[:], msk_sb[:],
                                start=False, stop=True,
                            )
                    nc.scalar.activation(
                        pt[:], sp[:], mybir.ActivationFunctionType.Exp,
                        scale=float(SCALE),
                    )
                    pts_c.append(pt)
                pts_all.append(pts_c)

                # previous chunk's PV chains: their exps are long done, so
                # the PE streams them while this chunk's exps run.
                if qc > 0:
                    emit_chains(qc - 1)

                # --- vT + V natural via PE transposes into slotted bank ---
                vps = psq.tile([128, CH], dt.float32, tag="pq")
                for cc in range(CC):
                    nc.tensor.matmul(
                        vps[:], wv_sb[:, cc, :], xt[:, cc, :],
                        start=(cc == 0), stop=(cc == CC - 1),
                    )
                vt = vtp.tile([128, CH], a_dt)
                nc.vector.tensor_copy(vt[:], vps[:])

                for ti in range(4):
                    jt = qc * 4 + ti
                    dst = tr_acc[:, ti * 128 : (ti + 1) * 128]
                    nc.tensor.transpose(
                        dst, vt[:, ti * 128 : (ti + 1) * 128], eye_sb[:]
                    )
                    nc.vector.tensor_copy(v_sb[:, jt, 0:H], dst)

            emit_chains(NCH - 1)

    nc.compile()
    return nc


def _get_nc():
    if "nc" not in _CACHE:
        _CACHE["nc"] = _build()
    return _CACHE["nc"]


def _np_dt(name):
    if name == "bfloat16":
        import ml_dtypes

        return ml_dtypes.bfloat16
    return np.float32


def _in_maps(x, Wq, Wk, Wv):
    pdt = _np_dt(PROJ_DT)
    adt = _np_dt(ATT_DT)

    def _wprep(W):
        # W [H, D] -> [128p, CC, H] with per-partition-contiguous rows
        WT = np.asarray(W, dtype=np.float32).T.reshape(CC, 128, H)
        return np.ascontiguousarray(WT.transpose(1, 0, 2)).astype(pdt)

    wq, wk, wv = _wprep(Wq), _wprep(Wk), _wprep(Wv)
    # msk[j, q] = MASKV where q < j (strictly masked in the diagonal block)
    msk = (MASKV * np.tril(np.ones((128, 128), dtype=np.float32), -1)).astype(adt)
    eye = np.eye(128, dtype=np.float32).astype(adt)
    x = np.asarray(x, dtype=np.float32)
    maps = []
    for b in range(B):
        # [qc, p, cc, t]: per (qc, p) a contiguous CC*CH run
        xh = np.ascontiguousarray(
            x[b].T.reshape(CC, 128, NCH, CH).transpose(2, 1, 0, 3)
        ).astype(pdt)
        maps.append(
            {
                "xh": xh, "wqT": wq, "wkT": wk, "wvT": wv,
                "msk": msk, "eye": eye,
            }
        )
    return maps


def kernel(x, Wq, Wk, Wv):
    from concourse.bass_utils import run_bass_kernel_spmd

    nc = _get_nc()
    res = run_bass_kernel_spmd(nc, _in_maps(x, Wq, Wk, Wv), core_ids=list(range(B)))
    return np.stack([res.results[b]["out"] for b in range(B)]).astype(np.float32)
